# revision 78
# baseline (speedup 1.0000x reference)
"""AdderVDSR kernel for 8 TRN2 NeuronCores.

Mathematical collapse: every AdderNet block computes
    relu(-sum_{c,kh,kw} |patch - w|)
and the inner sum of 576 absolute values of continuous random quantities is
strictly positive, so each block outputs exactly 0 in fp32.  After the first
adder layer the hidden state is identically zero and stays zero, so

    reference(x, ...) == pixel_shuffle(conv3(x, up_w, up_b), 2) + out_b

bit-exactly (conv3 of a zero tensor is exactly zero; relu of a negative
number is exactly 0.0).  The kernel therefore only computes the 3->12 channel
3x3 up-conv, the pixel shuffle, and the two bias adds.

Distribution: data-parallel over H.  Core i computes pre-shuffle rows
[16*i, 16*i+16) -> output rows [32*i, 32*i+32).  The host shards x into
per-core im2col patch tensors in bf16 (layout replication only; all FLOPs
run on device; fp32 PSUM accumulate keeps rel err ~2.5e-3).  On device: one
bf16 matmul per (row-half, parity, batch, bank-pair); biases fold in via a
ones row; the pixel-shuffle column interleave happens in the PSUM->SBUF
stage (stride-2 destinations, VectorE b=0 / ScalarE b=1) pipelined behind
the matmuls at half-slab granularity; f32 output leaves over three DMA
paths (SP ring / ACT ring / Pool SWDGE) as 12 half-slab transfers.
"""

import numpy as np

import concourse.bass as bass
import concourse.mybir as mybir
from concourse.bass_utils import run_bass_kernel_spmd

N_CORES = 8
B, C, H, W = 2, 3, 128, 128
RH = H // N_CORES          # 16 pre-shuffle rows per core
NPIX = B * RH * W          # 4096 pre-shuffle pixels per core
OC = 12                    # up-conv output channels (= 4*C)
K = 28                     # im2col contraction: 27 taps + ones row (bias)
XW = NPIX + 16             # xcol width: patches + packed weight columns

_f32 = mybir.dt.float32
_bf16 = mybir.dt.bfloat16


def build_graph():
    nc = bass.Bass()
    xcol = nc.declare_dram_parameter("xcol", [K, XW], _bf16, isOutput=False)
    out = nc.declare_dram_parameter("out", [B, C, 2 * RH, 2 * W], _f32, isOutput=True)

    with (
        nc.sbuf_tensor([K, XW + 2], _bf16) as P,
        nc.sbuf_tensor([38, NPIX + 16], _f32) as sb_out,
        nc.psum_tensor([38, NPIX // 4], _f32) as pse_h0,
        nc.psum_tensor([38, NPIX // 4], _f32) as pso_h0,
        nc.psum_tensor([38, NPIX // 4], _f32) as pse_h1,
        nc.psum_tensor([38, NPIX // 4], _f32) as pso_h1,
        nc.semaphore("dma_in") as dma_in,
        nc.semaphore("dma_canary") as dma_canary,
        nc.semaphore("dma_in2") as dma_in2,
        nc.semaphore("mm_sem") as mm_sem,
        nc.semaphore("cp0") as cp0,
        nc.semaphore("cp1") as cp1,
        nc.semaphore("dma_out_sem") as dma_out_sem,
        nc.semaphore("dma_out2_sem") as dma_out2_sem,
        nc.Block() as block,
    ):
        # xcol column layout: [wb (16) | b=0 patches (2048) | b=1 patches (2048)]
        def wslice(dc):
            return P[:, 6 * dc : 6 * dc + 6]

        def rhslice(b, rq):
            lo = 16 + b * (RH * W) + rq * 512
            return P[0:K, lo : lo + 512]

        def out_dma(eng, b, c, h, sem):
            # Half-slab h covers pre-shuffle rows [8h, 8h+8) -> output rows
            # [16h, 16h+16).
            src = sb_out[
                32 * b + 2 * c : 32 * b + 2 * c + 2, 2048 * h : 2048 * (h + 1)
            ].rearrange("dr (r col) -> dr r col", r=RH // 2, col=2 * W)
            dst = out[b, c, 16 * h : 16 * (h + 1), :].rearrange(
                "(r dr) col -> dr r col", dr=2
            )
            return eng.dma_start(out=dst, in_=src).then_inc(sem, 16)

        @block.sync
        def _(sync):
            # Input in 2 chunks: the b=0 matmuls start while the b=1 half's
            # completion latency hides behind them: [weights + b0 | b1].
            s2 = 16 + RH * W
            sync.dma_start(out=P[:, :s2], in_=xcol[:, :s2]).then_inc(dma_in, 16)
            # Canary: same partitions (= same SDMA engines), queued right
            # behind chunk 1 on the same ring.  Per-engine FIFO means its
            # completion implies chunk 1's data landed; its single-descriptor
            # completion semaphore may post faster than the big chunk's.
            sync.dma_start(out=P[:, XW : XW + 2], in_=xcol[:, 0:2]).then_inc(
                dma_canary, 16
            )
            sync.dma_start(out=P[:, s2:XW], in_=xcol[:, s2:]).then_inc(dma_in2, 16)
            # Half-slab output DMAs spread over three issuers (each dma_start
            # occupies its issuing engine for the whole transfer): SP takes
            # the c=2 slabs, Pool b=0 c=0,1, ACT b=1 c=0,1.
            for h in range(2):
                sync.wait_ge(cp0, h + 1)
                out_dma(sync, 0, 2, h, dma_out_sem)
                sync.wait_ge(cp1, h + 1)
                out_dma(sync, 1, 2, h, dma_out_sem)
            sync.wait_ge(dma_out_sem, 128)
            sync.wait_ge(dma_out2_sem, 64)

        @block.gpsimd
        def _(gpsimd):
            # b=0 c=0,1 half-slabs via SWDGE on the otherwise-idle Pool engine.
            for h in range(2):
                gpsimd.wait_ge(cp0, h + 1)
                for c in range(2):
                    out_dma(gpsimd, 0, c, h, dma_out2_sem)

        @block.tensor
        def _(tensor):
            # Half-outer, parity-next order: copies of quad (h, dc) start as
            # soon as its 4 matmuls retire, while the PE moves on to other
            # quads.  Each quad owns its own PSUM bank pair, so a PE write
            # never shares a bank with a concurrent DVE/ACT read.
            # Partition 32*b + (c*2+dr); slot (rq-2h)*512 + r*W + col.
            quads = ((pse_h0, pso_h0), (pse_h1, pso_h1))
            for h in range(2):
                for dc in range(2):
                    pst = quads[h][dc]
                    for b in range(B):
                        pb = 32 * b
                        for rq in (2 * h, 2 * h + 1):
                            if h == 0 and dc == 0 and rq == 0:
                                tensor.wait_ge(dma_canary if b == 0 else dma_in2, 16)
                            o = pst[pb : pb + 6, (rq - 2 * h) * 512 : (rq - 2 * h + 1) * 512]
                            mm = tensor.matmul(
                                o, lhsT=wslice(dc), rhs=rhslice(b, rq),
                                start=True, stop=True,
                            )
                    mm.then_inc(mm_sem, 1)

        # PSUM -> SBUF staging with the pixel-shuffle column interleave
        # (stride-2 destinations).  Lane-aligned; VectorE takes b=0 while
        # ScalarE takes b=1.
        @block.vector
        def _(vector):
            for h in range(2):
                for dc in range(2):
                    pst = ((pse_h0, pso_h0), (pse_h1, pso_h1))[h][dc]
                    vector.wait_ge(mm_sem, 2 * h + dc + 1)
                    cp = vector.tensor_copy(
                        sb_out[0:6, 2048 * h + dc : 2048 * (h + 1) : 2], pst[0:6, :]
                    )
                    if dc == 1:
                        cp.then_inc(cp0, 1)

        @block.scalar
        def _(scalar):
            # Dummy tiny copy: pulls the ACT_TABLE_LOAD for Copy forward,
            # off the post-matmul critical path.
            scalar.wait_ge(dma_in, 16)
            scalar.copy(sb_out[32:33, NPIX : NPIX + 16], P[0:1, 0:16])
            for h in range(2):
                for dc in range(2):
                    pst = ((pse_h0, pso_h0), (pse_h1, pso_h1))[h][dc]
                    scalar.wait_ge(mm_sem, 2 * h + dc + 1)
                    cp = scalar.copy(
                        sb_out[32:38, 2048 * h + dc : 2048 * (h + 1) : 2], pst[32:38, :]
                    )
                    if dc == 1:
                        cp.then_inc(cp1, 1)
                # b=1 c=0,1 half-slabs on the ACT HWDGE ring.  Self-wait: the
                # DMA must not read sb_out before the deep ACT pipeline has
                # retired the copies.
                scalar.wait_ge(cp1, h + 1)
                for c in range(2):
                    out_dma(scalar, 1, c, h, dma_out_sem)

    return nc


def make_in_maps(x, up_w, up_b, out_b):
    """Shard inputs: per-core im2col patches with packed weight columns."""
    import ml_dtypes

    bf16 = ml_dtypes.bfloat16
    x = np.asarray(x, dtype=np.float32)
    up_w = np.asarray(up_w, dtype=np.float32)
    up_b = np.asarray(up_b, dtype=np.float32)
    out_b = np.asarray(out_b, dtype=np.float32)

    # wb[c2*9+kh*3+kw, 6*dc + (c*2+dr)] = up_w[c*4+dr*2+dc, c2, kh, kw]
    # wb[27, 6*dc + (c*2+dr)] = up_b[o] + out_b[c]
    wb = np.zeros((K, 16), dtype=np.float32)
    for c in range(C):
        for dr in range(2):
            for dc in range(2):
                o = c * 4 + dr * 2 + dc
                col = 6 * dc + c * 2 + dr
                wb[:27, col] = up_w[o].reshape(27)
                wb[27, col] = up_b[o] + out_b[c]

    xp = np.zeros((B, C, H + 2, W + 2), dtype=np.float32)
    xp[:, :, 1 : H + 1, 1 : W + 1] = x

    in_maps = []
    for i in range(N_CORES):
        xcol = np.empty((K, XW), dtype=np.float32)
        pat = xcol[:, 16:].reshape(K, B, RH, W)
        for c in range(C):
            for kh in range(3):
                for kw in range(3):
                    k = c * 9 + kh * 3 + kw
                    pat[k] = xp[:, c, 16 * i + kh : 16 * i + kh + RH, kw : kw + W]
        pat[27] = 1.0
        xcol[:, :16] = wb
        in_maps.append({"xcol": xcol.astype(bf16)})
    return in_maps


def kernel(x, up_w, up_b, in_w, in_b, adder_w, out_w, out_b):
    nc = build_graph()
    in_maps = make_in_maps(x, up_w, up_b, out_b)
    res = run_bass_kernel_spmd(nc, in_maps, core_ids=list(range(N_CORES)))
    slabs = [np.asarray(res.results[i]["out"]) for i in range(N_CORES)]
    return np.concatenate(slabs, axis=2).astype(np.float32)

